# revision 10
# baseline (speedup 1.0000x reference)
"""Chamfer L1 loss (pytorch3d-style, norm=1, mean/mean reduction) on 8 Trainium2
NeuronCores via Bass/Tile.

Problem: mesh_x [4,4096,3], mesh_y [4,4096,3] (f32) ->
    loss = mean_i min_j d(x_i,y_j) + mean_j min_i d(x_i,y_j),  d = L1 distance.

Sharding: core c handles batch b = c//2 and x-row half h = c%2 (2048
x-points) against all 4096 y-points of that batch.  Per core, 16 tiles of
128 x-points (x on partitions, y on the free axis):
  - t_k = |y_k - x_k| per coordinate: ACT Abs(y*1 + bias) with the
    per-partition bias = -x, or on DVE as one tensor_scalar
    (add bias, then bitwise_and 0x7FFFFFFF clears the fp32 sign bit).
    y broadcast stays f32; t tiles are bf16 (rel err ~5e-5 measured).
  - d = (t0 + t1) + t2  (DVE tensor_tensor, bf16 2x mode)
  - x-direction min: fold d 4096->2048->1024->512 with bf16 2x
    tensor_tensor mins, then one small 1x tensor_reduce.
  - y-direction: ymin = min(ymin, d) accumulated across tiles.
Host side does the trivial unshard: sum of x-mins, 128-partition +
cross-core min of the y-partials, then the two means.
"""

import numpy as np
from contextlib import ExitStack

B = 4
N = 4096
M = 4096
P = 128
NCORES = 8
XTILES = (N // 2) // P  # 16 x-tiles of 128 rows per core

_BIG = 3.0e38

# Which t2-abs ops run on DVE (balance ACT vs DVE); pattern over tile idx.
ABS_DVE_EVERY = 2  # t % ABS_DVE_EVERY == 0 -> t2 abs on DVE
ABS_DVE_FUSED = False  # fused (add, bitwise_and) rejected by walrus on gen3
YMIN_DMA = False  # SWDGE dma accum_op rejected by walrus on this stack
POOL_YMIN_EVERY = 0  # >0: tiles with t % POOL_YMIN_EVERY == 2 do ymin on GPSIMD
REPEAT = 1  # replicate compute body (for timing; results are idempotent)


def _build_bass():
    import concourse.bass as bass  # noqa: F401
    import concourse.tile as tile
    from concourse import bacc, mybir

    f32 = mybir.dt.float32
    bf16 = mybir.dt.bfloat16
    u32 = mybir.dt.uint32
    Abs = mybir.ActivationFunctionType.Abs
    Alu = mybir.AluOpType

    nc = bacc.Bacc("TRN2", target_bir_lowering=False, num_devices=NCORES)

    ybc_d = nc.dram_tensor("ybc", [P, 3 * M], f32, kind="ExternalInput").ap()
    xneg_d = nc.dram_tensor("xneg", [P, 3 * XTILES], f32, kind="ExternalInput").ap()
    xmin_d = nc.dram_tensor("xmin", [P, XTILES], f32, kind="ExternalOutput").ap()
    ymin_d = nc.dram_tensor("ymin", [P, M], bf16, kind="ExternalOutput").ap()

    with tile.TileContext(nc) as tc:
        with ExitStack() as ctx:
            const = ctx.enter_context(tc.tile_pool(name="const", bufs=1))
            tpool = ctx.enter_context(tc.tile_pool(name="t", bufs=3))
            fpool = ctx.enter_context(tc.tile_pool(name="f", bufs=3))

            xn = const.tile([P, 3 * XTILES], f32, tag="xneg")
            nc.sync.dma_start(xn[:], xneg_d[:])
            y = []
            for k in range(3):
                yk = const.tile([P, M], f32, tag=f"y{k}")
                nc.sync.dma_start(yk[:], ybc_d[:, k * M : (k + 1) * M])
                y.append(yk)

            ymin = const.tile([P, M], bf16, tag="ymin")
            xmin = const.tile([P, XTILES], f32, tag="xmin")

            for _ in range(REPEAT):
                for t in range(XTILES):
                    c0 = xn[:, 3 * t : 3 * t + 1]
                    c1 = xn[:, 3 * t + 1 : 3 * t + 2]
                    c2 = xn[:, 3 * t + 2 : 3 * t + 3]

                    t0 = tpool.tile([P, M], bf16, tag="t0")
                    nc.scalar.activation(t0[:], y[0][:], Abs, bias=c0, scale=1.0)
                    t1 = tpool.tile([P, M], bf16, tag="t1")
                    nc.scalar.activation(t1[:], y[1][:], Abs, bias=c1, scale=1.0)
                    t01 = tpool.tile([P, M], bf16, tag="t01")
                    nc.vector.tensor_tensor(t01[:], t0[:], t1[:], Alu.add)

                    t2 = tpool.tile([P, M], bf16, tag="t2")
                    if t % ABS_DVE_EVERY == 0:
                        if ABS_DVE_FUSED:
                            nc.vector.tensor_scalar(
                                t2[:], y[2][:], c2, 0x7FFFFFFF, Alu.add, Alu.bitwise_and
                            )
                        else:
                            nc.vector.tensor_scalar(t2[:], y[2][:], c2, None, Alu.add)
                            t2i = t2[:].bitcast(u32)
                            nc.vector.tensor_scalar(
                                t2i, t2i, 0x7FFF7FFF, None, Alu.bitwise_and
                            )
                    else:
                        nc.scalar.activation(t2[:], y[2][:], Abs, bias=c2, scale=1.0)

                    d = tpool.tile([P, M], bf16, tag="d")
                    nc.vector.tensor_tensor(d[:], t01[:], t2[:], Alu.add)

                    # y-direction partial mins (first tile: plain copy, 4x mode)
                    if t == 0:
                        nc.vector.tensor_copy(ymin[:], d[:])
                    elif YMIN_DMA:
                        nc.gpsimd.dma_start(ymin[:], d[:], accum_op=Alu.min)
                    elif POOL_YMIN_EVERY and t % POOL_YMIN_EVERY == 2:
                        nc.gpsimd.tensor_tensor(ymin[:], ymin[:], d[:], Alu.min)
                    else:
                        nc.vector.tensor_tensor(ymin[:], ymin[:], d[:], Alu.min)

                    # x-direction min: fold 4096->512 at bf16 2x, then reduce
                    f1 = fpool.tile([P, M // 2], bf16, tag="f1")
                    nc.vector.tensor_tensor(
                        f1[:], d[:, 0 : M // 2], d[:, M // 2 : M], Alu.min
                    )
                    f2 = fpool.tile([P, M // 4], bf16, tag="f2")
                    nc.vector.tensor_tensor(
                        f2[:], f1[:, 0 : M // 4], f1[:, M // 4 : M // 2], Alu.min
                    )
                    f3 = fpool.tile([P, M // 8], bf16, tag="f3")
                    nc.vector.tensor_tensor(
                        f3[:], f2[:, 0 : M // 8], f2[:, M // 8 : M // 4], Alu.min
                    )
                    nc.vector.tensor_reduce(
                        xmin[:, t : t + 1], f3[:], mybir.AxisListType.X, Alu.min
                    )

            nc.sync.dma_start(xmin_d[:], xmin[:])
            nc.sync.dma_start(ymin_d[:], ymin[:])

    nc.compile()
    return nc


LAST_PERF = None


def _shard_inputs(mesh_x, mesh_y):
    x = np.ascontiguousarray(np.asarray(mesh_x, dtype=np.float32))
    yy = np.ascontiguousarray(np.asarray(mesh_y, dtype=np.float32))
    in_maps = []
    for c in range(NCORES):
        b, h = divmod(c, 2)
        xs = x[b, h * (N // 2) : (h + 1) * (N // 2)]  # [2048, 3]
        # xneg[p, 3*t + k] = -xs[t*128 + p, k]
        xn = -xs.reshape(XTILES, P, 3).transpose(1, 0, 2).reshape(P, 3 * XTILES)
        # ybc[p, k*M + j] = y[b, j, k]
        ybc = np.broadcast_to(yy[b].T.reshape(1, 3 * M), (P, 3 * M))
        in_maps.append(
            {"ybc": np.ascontiguousarray(ybc), "xneg": np.ascontiguousarray(xn)}
        )
    return in_maps


def kernel(mesh_x: np.ndarray, mesh_y: np.ndarray) -> np.ndarray:
    global LAST_PERF
    from concourse.bass_utils import run_bass_kernel_spmd

    in_maps = _shard_inputs(mesh_x, mesh_y)
    nc = _build_bass()
    kr = run_bass_kernel_spmd(nc, in_maps, core_ids=list(range(NCORES)))
    LAST_PERF = kr
    res = kr.results

    sum_x = 0.0
    ymins = []
    for c in range(NCORES):
        sum_x += np.asarray(res[c]["xmin"], dtype=np.float64).sum()
        ymins.append(np.asarray(res[c]["ymin"], dtype=np.float32).min(axis=0))
    sum_y = 0.0
    for b in range(B):
        sum_y += np.minimum(ymins[2 * b], ymins[2 * b + 1]).sum(dtype=np.float64)

    loss = sum_x / (B * N) + sum_y / (B * M)
    return np.array(loss, dtype=np.float32)


# revision 11
# speedup vs baseline: 1.0709x; 1.0709x over previous
"""Chamfer L1 loss (pytorch3d-style, norm=1, mean/mean reduction) on 8 Trainium2
NeuronCores via Bass/Tile.

Problem: mesh_x [4,4096,3], mesh_y [4,4096,3] (f32) ->
    loss = mean_i min_j d(x_i,y_j) + mean_j min_i d(x_i,y_j),  d = L1 distance.

Sharding: core c handles batch b = c//2 and x-row half h = c%2 (2048
x-points) against all 4096 y-points of that batch.  Per core, 16 tiles of
128 x-points (x on partitions, y on the free axis):
  - t_k = |y_k - x_k| per coordinate: ACT Abs(y*1 + bias) with the
    per-partition bias = -x, or on DVE as one tensor_scalar
    (add bias, then bitwise_and 0x7FFFFFFF clears the fp32 sign bit).
    y broadcast stays f32; t tiles are bf16 (rel err ~5e-5 measured).
  - d = (t0 + t1) + t2  (DVE tensor_tensor, bf16 2x mode)
  - x-direction min: fold d 4096->2048->1024->512 with bf16 2x
    tensor_tensor mins, then one small 1x tensor_reduce.
  - y-direction: ymin = min(ymin, d) accumulated across tiles.
Host side does the trivial unshard: sum of x-mins, 128-partition +
cross-core min of the y-partials, then the two means.
"""

import numpy as np
from contextlib import ExitStack

B = 4
N = 4096
M = 4096
P = 128
NCORES = 8
XTILES = (N // 2) // P  # 16 x-tiles of 128 rows per core

_BIG = 3.0e38

# Which t2-abs ops run on DVE (balance ACT vs DVE); pattern over tile idx.
ABS_DVE_EVERY = 4  # t % ABS_DVE_EVERY == 0 -> t2 abs on DVE
ABS_DVE_FUSED = False  # fused (add, bitwise_and) rejected by walrus on gen3
YMIN_DMA = False  # SWDGE dma accum_op rejected by walrus on this stack
POOL_YMIN_EVERY = 0  # >0: tiles with t % POOL_YMIN_EVERY == 2 do ymin on GPSIMD
REPEAT = 1  # replicate compute body (for timing; results are idempotent)


def _build_bass():
    import concourse.bass as bass  # noqa: F401
    import concourse.tile as tile
    from concourse import bacc, mybir

    f32 = mybir.dt.float32
    bf16 = mybir.dt.bfloat16
    u32 = mybir.dt.uint32
    Abs = mybir.ActivationFunctionType.Abs
    Alu = mybir.AluOpType

    nc = bacc.Bacc("TRN2", target_bir_lowering=False, num_devices=NCORES)

    ybc_d = nc.dram_tensor("ybc", [P, 3 * M], f32, kind="ExternalInput").ap()
    xneg_d = nc.dram_tensor("xneg", [P, 3 * XTILES], f32, kind="ExternalInput").ap()
    xmin_d = nc.dram_tensor("xmin", [P, XTILES], f32, kind="ExternalOutput").ap()
    ymin_d = nc.dram_tensor("ymin", [P, M], bf16, kind="ExternalOutput").ap()

    with tile.TileContext(nc) as tc:
        with ExitStack() as ctx:
            const = ctx.enter_context(tc.tile_pool(name="const", bufs=1))
            tpool = ctx.enter_context(tc.tile_pool(name="t", bufs=3))
            fpool = ctx.enter_context(tc.tile_pool(name="f", bufs=3))

            xn = const.tile([P, 3 * XTILES], f32, tag="xneg")
            nc.sync.dma_start(xn[:], xneg_d[:])
            y = []
            for k in range(3):
                yk = const.tile([P, M], f32, tag=f"y{k}")
                nc.sync.dma_start(yk[:], ybc_d[:, k * M : (k + 1) * M])
                y.append(yk)

            ymin = const.tile([P, M], bf16, tag="ymin")
            xmin = const.tile([P, XTILES], f32, tag="xmin")

            for _ in range(REPEAT):
                for t in range(XTILES):
                    c0 = xn[:, 3 * t : 3 * t + 1]
                    c1 = xn[:, 3 * t + 1 : 3 * t + 2]
                    c2 = xn[:, 3 * t + 2 : 3 * t + 3]

                    t0 = tpool.tile([P, M], bf16, tag="t0")
                    nc.scalar.activation(t0[:], y[0][:], Abs, bias=c0, scale=1.0)
                    t1 = tpool.tile([P, M], bf16, tag="t1")
                    nc.scalar.activation(t1[:], y[1][:], Abs, bias=c1, scale=1.0)
                    t01 = tpool.tile([P, M], bf16, tag="t01")
                    nc.vector.tensor_tensor(t01[:], t0[:], t1[:], Alu.add)

                    t2 = tpool.tile([P, M], bf16, tag="t2")
                    if t % ABS_DVE_EVERY == 0:
                        if ABS_DVE_FUSED:
                            nc.vector.tensor_scalar(
                                t2[:], y[2][:], c2, 0x7FFFFFFF, Alu.add, Alu.bitwise_and
                            )
                        else:
                            nc.vector.tensor_scalar(t2[:], y[2][:], c2, None, Alu.add)
                            t2i = t2[:].bitcast(u32)
                            nc.vector.tensor_scalar(
                                t2i, t2i, 0x7FFF7FFF, None, Alu.bitwise_and
                            )
                    else:
                        nc.scalar.activation(t2[:], y[2][:], Abs, bias=c2, scale=1.0)

                    d = tpool.tile([P, M], bf16, tag="d")
                    nc.vector.tensor_tensor(d[:], t01[:], t2[:], Alu.add)

                    # y-direction partial mins (first tile: plain copy, 4x mode)
                    if t == 0:
                        nc.vector.tensor_copy(ymin[:], d[:])
                    elif YMIN_DMA:
                        nc.gpsimd.dma_start(ymin[:], d[:], accum_op=Alu.min)
                    elif POOL_YMIN_EVERY and t % POOL_YMIN_EVERY == 2:
                        nc.gpsimd.tensor_tensor(ymin[:], ymin[:], d[:], Alu.min)
                    else:
                        nc.vector.tensor_tensor(ymin[:], ymin[:], d[:], Alu.min)

                    # x-direction min: fold 4096->512 at bf16 2x, then reduce
                    f1 = fpool.tile([P, M // 2], bf16, tag="f1")
                    nc.vector.tensor_tensor(
                        f1[:], d[:, 0 : M // 2], d[:, M // 2 : M], Alu.min
                    )
                    f2 = fpool.tile([P, M // 4], bf16, tag="f2")
                    nc.vector.tensor_tensor(
                        f2[:], f1[:, 0 : M // 4], f1[:, M // 4 : M // 2], Alu.min
                    )
                    f3 = fpool.tile([P, M // 8], bf16, tag="f3")
                    nc.vector.tensor_tensor(
                        f3[:], f2[:, 0 : M // 8], f2[:, M // 8 : M // 4], Alu.min
                    )
                    nc.vector.tensor_reduce(
                        xmin[:, t : t + 1], f3[:], mybir.AxisListType.X, Alu.min
                    )

            nc.sync.dma_start(xmin_d[:], xmin[:])
            nc.sync.dma_start(ymin_d[:], ymin[:])

    nc.compile()
    return nc


LAST_PERF = None


def _shard_inputs(mesh_x, mesh_y):
    x = np.ascontiguousarray(np.asarray(mesh_x, dtype=np.float32))
    yy = np.ascontiguousarray(np.asarray(mesh_y, dtype=np.float32))
    in_maps = []
    for c in range(NCORES):
        b, h = divmod(c, 2)
        xs = x[b, h * (N // 2) : (h + 1) * (N // 2)]  # [2048, 3]
        # xneg[p, 3*t + k] = -xs[t*128 + p, k]
        xn = -xs.reshape(XTILES, P, 3).transpose(1, 0, 2).reshape(P, 3 * XTILES)
        # ybc[p, k*M + j] = y[b, j, k]
        ybc = np.broadcast_to(yy[b].T.reshape(1, 3 * M), (P, 3 * M))
        in_maps.append(
            {"ybc": np.ascontiguousarray(ybc), "xneg": np.ascontiguousarray(xn)}
        )
    return in_maps


def kernel(mesh_x: np.ndarray, mesh_y: np.ndarray) -> np.ndarray:
    global LAST_PERF
    from concourse.bass_utils import run_bass_kernel_spmd

    in_maps = _shard_inputs(mesh_x, mesh_y)
    nc = _build_bass()
    kr = run_bass_kernel_spmd(nc, in_maps, core_ids=list(range(NCORES)))
    LAST_PERF = kr
    res = kr.results

    sum_x = 0.0
    ymins = []
    for c in range(NCORES):
        sum_x += np.asarray(res[c]["xmin"], dtype=np.float64).sum()
        ymins.append(np.asarray(res[c]["ymin"], dtype=np.float32).min(axis=0))
    sum_y = 0.0
    for b in range(B):
        sum_y += np.minimum(ymins[2 * b], ymins[2 * b + 1]).sum(dtype=np.float64)

    loss = sum_x / (B * N) + sum_y / (B * M)
    return np.array(loss, dtype=np.float32)
